# revision 8
# baseline (speedup 1.0000x reference)
"""Trainium2 Bass kernel for the 4-branch "Memory multimode" attention module.

Reference computation (per batch element b):
    q  = q_key[b].reshape(1024, 128)        (row-major reinterpret)
    pq = p_q_key[b].reshape(1024, 128)
    k  = m_key[b].reshape(128, 2048)
    pk = p_m_key[b].reshape(128, 2048)
    mval = m_val[b].reshape(512, 2048).T    # [2048, 512]
    out  = (sm(q@k) + sm(pq@pk) + sm(pq@k) + sm(q@pk)) @ mval
    where sm() is softmax over the QUERY dim (axis 0 of each [1024, 2048] score
    matrix).  Final output channel-concats q_val.

Key algebraic point: all four branches share the same value matrix, so the four
softmax matrices are summed BEFORE the value matmul — one [1024,2048]@[2048,512]
matmul instead of four (2.5x FLOP reduction vs the naive form).

Implementation strategy (one NeuronCore per batch element, 8 cores):
  * Work in the transposed score layout S^T = [key_pos(l) x query(i)] so the
    softmax reduction runs along the DVE/ACT free dimension.
    S^T tiles come straight out of the PE: lhsT = keys[:, l-tile] (natural
    layout!), rhs = Q^T chunks (host pre-transposed).
  * Softmax over queries needs no max-subtraction: |scores| <= ~75 and
    exp(75) ~ 3.7e32 stays comfortably inside fp32.  The ScalarE exp pass uses
    accum_out to produce the denominators D (row sums) in the same pass.
  * The per-key-row 1/D scaling AND the 4-way branch sum run on the PE as
    accumulating diag(invD) @ E^T matmuls (cheap at float32r rates) - DVE
    would need ~8M 1x-mode elements otherwise.
  * float32r (replicated fp32) matmuls: full PE rate at N>=512 vs 4 cycles/row
    for plain fp32, with ~2^-16 relative error - far below the fp32 envelope
    needed here.
  * q_val never touches the device: it is concatenated on the host.

Inputs per core (host-prepared, all fp32):
    mk  [128, 2048]  m_key[b]   (natural reinterpret)
    pmk [128, 2048]  p_m_key[b]
    qt  [128, 1024]  q_key[b].reshape(1024,128).T
    pqt [128, 1024]  p_q_key[b].reshape(1024,128).T
    mvt [2048, 512]  m_val[b].reshape(512,2048).T
Output per core:
    out [1024, 512]  attention result matrix (row-major == [512,32,32] block)
"""

import numpy as np

import concourse.bass as bass
import concourse.mybir as mybir
import concourse.tile as tile
from concourse.bass_utils import run_bass_kernel_spmd
from concourse.masks import make_identity
from concourse.vector_clock import ScopedClock

# The walrus build in this image encodes Drain with the compact CTRL_NO_STRUCT
# format, which holds at most 2 sync-wait commands; TileContext's kernel-tail
# drain attaches one wait per outstanding proc-sem (3+ in any real kernel) and
# codegen dies with "Too many sync wait commands".  Split the waits across
# several consecutive drains on the sync engine instead.
_MAX_DRAIN_WAITS = 1


def _split_drain_and_barrier(self, tick_clock, wait_clock):
    nc = self.nc
    drain_inst = nc.sync.drain()
    wait_clock.add_sem_waits(
        drain_inst.ins, ScopedClock({None: tick_clock.global_clock})
    )
    mi = drain_inst.ins
    waits = list(mi.sync_info.on_wait)
    if len(waits) > _MAX_DRAIN_WAITS:
        del mi.sync_info.on_wait[_MAX_DRAIN_WAITS:]
        rest = waits[_MAX_DRAIN_WAITS:]
        for i in range(0, len(rest), _MAX_DRAIN_WAITS):
            extra = nc.sync.drain()
            if extra.ins.sync_info is None:
                extra.ins.sync_info = mybir.SyncInfo(on_wait=[], on_update=[])
            extra.ins.sync_info.on_wait.extend(rest[i : i + _MAX_DRAIN_WAITS])

    nc.all_engine_barrier()
    assert self.sems is not None
    popped = nc._tile_sem_poison_stack.pop()
    assert popped is self._sem_poison
    nc.clear_and_free_semaphores(list(self.sems.allocated().values()))
    nc.all_engine_barrier()


tile.TileContext._drain_and_barrier = _split_drain_and_barrier


def _split_sync_waits(nc, max_waits: int = _MAX_DRAIN_WAITS):
    """Walrus here caps sync-wait commands per instruction; Tile's wait
    assigner doesn't know that.  Move overflow waits onto NoOps inserted
    just before the over-subscribed instruction (same engine, same block)."""
    for f in nc.m.functions:
        for blk in f.blocks:
            insts = blk.instructions
            out = []
            changed = False
            for inst in insts:
                si = inst.sync_info
                cap = 1
                if si is not None and len(si.on_wait) > cap:
                    waits = list(si.on_wait)
                    rest, keep = waits[:-cap], waits[-cap:]
                    for i in range(0, len(rest), max(1, max_waits)):
                        noop = mybir.InstNoOp(
                            name=nc.get_next_instruction_name(), ins=[], outs=[]
                        )
                        noop.engine = inst.engine
                        noop.sync_info = mybir.SyncInfo(
                            on_wait=rest[i : i + max(1, max_waits)], on_update=[]
                        )
                        nc.register_instruction(noop)
                        out.append(noop)
                    inst.sync_info = mybir.SyncInfo(
                        on_wait=keep, on_update=list(si.on_update)
                    )
                    changed = True
                out.append(inst)
            if changed:
                blk.instructions = out
    return nc

B, H, W = 8, 32, 32
HW = H * W          # 1024 queries
KD = 128            # key dim
VD = 512            # val dim
L = 2 * HW          # 2048 key positions
NT = L // 128       # 16 l-tiles
NCORES = 8

F32 = mybir.dt.float32

_nc_cache = {}


def build_nc(use_f32r: bool = True):
    nc = bass.Bass("TRN2", target_bir_lowering=False, debug=False)
    mmdt = mybir.dt.float32r if use_f32r else mybir.dt.float32
    mk = nc.dram_tensor("mk", [KD, L], mmdt, kind="ExternalInput").ap()
    pmk = nc.dram_tensor("pmk", [KD, L], mmdt, kind="ExternalInput").ap()
    qt = nc.dram_tensor("qt", [KD, HW], mmdt, kind="ExternalInput").ap()
    pqt = nc.dram_tensor("pqt", [KD, HW], mmdt, kind="ExternalInput").ap()
    mvt = nc.dram_tensor("mvt", [L, VD], mmdt, kind="ExternalInput").ap()
    out = nc.dram_tensor("out", [HW, VD], F32, kind="ExternalOutput").ap()

    EXP = mybir.ActivationFunctionType.Exp

    with tile.TileContext(nc) as tc:
        with (
            tc.tile_pool(name="keys", bufs=1) as keys_pool,
            tc.tile_pool(name="qts", bufs=1) as qt_pool,
            tc.tile_pool(name="mv", bufs=1) as mv_pool,
            tc.tile_pool(name="ework", bufs=2) as e_pool,
            tc.tile_pool(name="atiles", bufs=1) as a_pool,
            tc.tile_pool(name="const", bufs=1) as const_pool,
            tc.tile_pool(name="dwork", bufs=3) as d_pool,
            tc.tile_pool(name="ostage", bufs=2) as out_pool,
            tc.tile_pool(name="psum_s", bufs=2, space="PSUM") as psum_s,
            tc.tile_pool(name="psum_a", bufs=1, space="PSUM") as psum_a,
            tc.tile_pool(name="psum_o", bufs=2, space="PSUM") as psum_o,
        ):
            # ---- input loads -------------------------------------------------
            keys = keys_pool.tile([128, 2 * L], mmdt)     # [128, 4096] = mk | pmk
            nc.sync.dma_start(keys[:, 0:L], mk)
            nc.sync.dma_start(keys[:, L : 2 * L], pmk)
            qts = qt_pool.tile([128, 2 * HW], mmdt)       # [128, 2048] = qt | pqt
            nc.sync.dma_start(qts[:, 0:HW], qt)
            nc.sync.dma_start(qts[:, HW : 2 * HW], pqt)
            mv_tiles = []
            for t in range(NT):
                mvtile = mv_pool.tile([128, VD], mmdt, tag=f"mv{t}")
                nc.sync.dma_start(mvtile[:], mvt[t * 128 : (t + 1) * 128, :])
                mv_tiles.append(mvtile)

            ident = const_pool.tile([128, 128], F32)
            make_identity(nc, ident)

            # ---- phase 1: scores -> exp(+rowsum) -> diag-scaled sum A^T ------
            a_tiles = []
            for t in range(NT):
                dtile = d_pool.tile([128, 4], F32, tag="D")
                e_tiles = []
                for y in range(2):  # 0 = keys from m_key, 1 = from p_m_key
                    e_t = e_pool.tile([128, 2 * HW], mmdt, tag=f"E{y}")
                    lhsT = keys[:, y * L + t * 128 : y * L + (t + 1) * 128]
                    for xh in range(2):  # 0 = q queries, 1 = pq queries
                        s_ps = psum_s.tile([128, HW], F32, tag="S")
                        for c in range(2):
                            rhs = qts[:, xh * HW + c * 512 : xh * HW + (c + 1) * 512]
                            nc.tensor.matmul(
                                s_ps[:, c * 512 : (c + 1) * 512],
                                lhsT,
                                rhs,
                                start=True,
                                stop=True,
                            )
                        # E^T = exp(S^T); accum_out = row sums = softmax denom
                        nc.scalar.activation(
                            e_t[:, xh * HW : (xh + 1) * HW],
                            s_ps[:],
                            EXP,
                            accum_out=dtile[:, 2 * y + xh : 2 * y + xh + 1],
                        )
                    e_tiles.append(e_t)

                invd = d_pool.tile([128, 4], F32, tag="invD")
                nc.vector.reciprocal(invd[:], dtile[:])
                diags = []
                for j in range(4):
                    dg = d_pool.tile([128, 128], mmdt, tag=f"diag{j}")
                    nc.vector.tensor_scalar_mul(dg[:], ident[:], invd[:, j : j + 1])
                    diags.append(dg)

                # A^T[t][l, i] = sum_{y,xh} invD[y,xh][l] * E^T_y[l, xh*HW + i]
                a_ps = psum_a.tile([128, HW], F32, tag="A")
                for ih in range(2):  # query-column half of A^T
                    n = 0
                    for y in range(2):
                        for xh in range(2):
                            rhs = e_tiles[y][
                                :, xh * HW + ih * 512 : xh * HW + (ih + 1) * 512
                            ]
                            nc.tensor.matmul(
                                a_ps[:, ih * 512 : (ih + 1) * 512],
                                diags[2 * y + xh][:],
                                rhs,
                                start=(n == 0),
                                stop=(n == 3),
                            )
                            n += 1
                a_sb = a_pool.tile([128, HW], mmdt, tag=f"A{t}")
                nc.vector.tensor_copy(a_sb[:], a_ps[:])
                a_tiles.append(a_sb)

            # ---- phase 2: out[i,:] = sum_l A^T[l,i] * mvt[l,:] ---------------
            for i in range(HW // 128):
                o_ps = psum_o.tile([128, VD], F32, tag="O")
                for t in range(NT):
                    nc.tensor.matmul(
                        o_ps[:],
                        a_tiles[t][:, i * 128 : (i + 1) * 128],
                        mv_tiles[t][:],
                        start=(t == 0),
                        stop=(t == NT - 1),
                    )
                o_sb = out_pool.tile([128, VD], F32, tag="osb")
                nc.vector.tensor_copy(o_sb[:], o_ps[:])
                nc.sync.dma_start(out[i * 128 : (i + 1) * 128, :], o_sb[:])

    _split_sync_waits(nc)
    return nc


def make_in_maps(m_key, m_val, q_key, p_m_key, p_q_key):
    in_maps = []
    for b in range(B):
        in_maps.append(
            {
                "mk": np.ascontiguousarray(m_key[b].reshape(KD, L)),
                "pmk": np.ascontiguousarray(p_m_key[b].reshape(KD, L)),
                "qt": np.ascontiguousarray(q_key[b].reshape(HW, KD).T),
                "pqt": np.ascontiguousarray(p_q_key[b].reshape(HW, KD).T),
                "mvt": np.ascontiguousarray(m_val[b].reshape(VD, L).T),
            }
        )
    return in_maps


def run(inputs, trace: bool = False, use_f32r: bool = True):
    """Run on the 8 NeuronCores; returns (full_output, BassKernelResults)."""
    inputs = {k: np.asarray(v, dtype=np.float32) for k, v in inputs.items()}
    key = use_f32r
    if key not in _nc_cache:
        _nc_cache[key] = build_nc(use_f32r)
    nc = _nc_cache[key]
    in_maps = make_in_maps(
        inputs["m_key"], inputs["m_val"], inputs["q_key"],
        inputs["p_m_key"], inputs["p_q_key"],
    )
    res = run_bass_kernel_spmd(nc, in_maps, list(range(NCORES)), trace=trace)
    q_val = inputs["q_val"]
    outs = []
    for b in range(B):
        mat = np.asarray(res.results[b]["out"])      # [1024, 512]
        attn = mat.reshape(VD, H, W)                 # row-major reinterpret
        outs.append(np.concatenate([attn, q_val[b]], axis=0))
    return np.stack(outs), res


def kernel(**inputs) -> np.ndarray:
    out, _ = run(inputs, trace=False)
    return out
